# revision 79
# baseline (speedup 1.0000x reference)
"""Sliding-window GQA attention on 8 TRN2 NeuronCores.

Sharding: core c handles batch b=c//4 and kv-head pair 2*(c%4)..+1
(-> 4 query heads, 2 kv heads, all 2048 tokens of one batch).
Each core computes its heads' partial o-projection [2048, 3584] in bf16;
the host sums the 4 partials per batch. No on-device collectives.

All heavy matmuls run in bf16 (fp32 PSUM accumulate). Host pre-lays-out
x^T / weights in [128-partition, ...] tiles so every DMA is contiguous.

Attention computes logits TRANSPOSED: logitsT[s,t] = matmul(lhsT=kT,
rhs=qT two heads wide), exp'd probs expT[s,t] feed the PV matmul
directly as the stationary operand (no PE transposes of probabilities).
The softmax denominator comes free from a ones-column appended to V;
SCALE/rms(q) is folded into qT during the projection epilogue. Two
s-blocks share each logits PSUM bank so one Exp covers both, QK runs
two pairs ahead of PV to hide the Exp latency, and phase 2 software-
pipelines att(tb) -> oproj(tb-1) -> transposes(tb) so every epilogue
chain hides under a dense o-proj. The last two K-projection epilogues
defer into phase 2 to keep the phase boundary tight, and dummy
identity matmuls hold the PE's HAM clock at 2.4 GHz through the
DMA-supply-bound startup.
"""

import os
import numpy as np
import ml_dtypes

B, T, D, H = 2, 2048, 3584, 256
QH, KVH = 4, 2          # per-core q heads / kv heads
DC = D // 128           # 28 contract chunks
DCQ = DC // 4           # 7 chunks per weight quarter
TBN = T // 128          # 16 token blocks
HC = H // 128           # 2 head-dim chunks
OC = QH * H // 128      # 8 out-proj contract chunks
SCALE = 0.0625
EPS = 1e-6
ROPE_BASE = 10000.0
WB = 1024 // 128        # window in blocks (8)
NEG = -1.0e30
NDOUT = D // 512        # 7 o-proj column chunks
VW = 264                # vS row width: 256 v cols + ones col at 256 + pad

BF16 = ml_dtypes.bfloat16

_cached = {}


def _build():
    import concourse.bass as bass
    import concourse.mybir as mybir
    import concourse.tile as tile
    from concourse import bacc
    from concourse.masks import make_identity

    f32 = mybir.dt.float32
    bf16 = mybir.dt.bfloat16
    AF = mybir.ActivationFunctionType

    nc = bacc.Bacc(None, target_bir_lowering=False)

    xT_d = nc.dram_tensor("xT", [128, TBN, DC, 128], bf16, kind="ExternalInput")
    wq_d = nc.dram_tensor("wq", [128, DC, QH * H], bf16, kind="ExternalInput")
    wkv_d = nc.dram_tensor("wkv", [128, DC, 2 * KVH * H], bf16, kind="ExternalInput")
    wo_d = nc.dram_tensor("wo", [128, OC, D], bf16, kind="ExternalInput")
    cos_d = nc.dram_tensor("cos", [128, TBN, 128], bf16, kind="ExternalInput")
    sin_d = nc.dram_tensor("sin", [128, TBN, 128], bf16, kind="ExternalInput")
    qsc_d = nc.dram_tensor("qsc", [128, HC], f32, kind="ExternalInput")
    ksc_d = nc.dram_tensor("ksc", [128, HC], f32, kind="ExternalInput")
    mdiag_d = nc.dram_tensor("mdiag", [128, 128], f32, kind="ExternalInput")
    medge_d = nc.dram_tensor("medge", [128, 128], f32, kind="ExternalInput")
    out_d = nc.dram_tensor("out", [T, D], bf16, kind="ExternalOutput")

    with tile.TileContext(nc) as tc:
        with (
            tc.tile_pool(name="persist", bufs=1) as pers,
            tc.tile_pool(name="wa", bufs=4) as wap,   # wq quarters, then wo
            tc.tile_pool(name="wb", bufs=4) as wbp,   # wkv quarters
        ):
            qT = pers.tile([128, HC, QH, T], bf16)     # q^T  [h, hc, head, t]
            kT = pers.tile([128, HC, KVH, T], bf16)    # k^T  [h, hc, kv, s]
            vS = pers.tile([128, TBN, KVH, VW], bf16)  # v    [s, sblock, kv, h]
            kqP = pers.tile([128, 2, KVH, H], bf16)    # tb14/15 K psum parked
            qsc = pers.tile([128, HC], f32)
            ksc = pers.tile([128, HC], f32)
            ident = pers.tile([128, 128], bf16)
            make_identity(nc, ident)
            epsb = pers.tile([128, 1], f32)
            epsb2 = pers.tile([128, 1], f32)
            mdiag = pers.tile([128, 128], f32)
            medge = pers.tile([128, 128], f32)
            # force the Exp activation table to load during the startup DMA
            # wait instead of at the first real softmax.
            wexp = pers.tile([128, 1], f32)
            nc.scalar.activation(wexp, epsb, AF.Exp)
            nc.scalar.activation(wexp, wexp, AF.Copy, scale=0.0)
            nc.vector.tensor_add(mdiag[:, 0:1], mdiag[:, 0:1], wexp)

            wq_p = [wap.tile([128, DCQ, QH * H], bf16, tag="wa", name=f"wq_{qi}")
                    for qi in range(4)]

            # ---------------- phase 1: projections ----------------
            with (
                tc.tile_pool(name="xt", bufs=3) as pxt,
                tc.tile_pool(name="scr", bufs=3) as scr,
                tc.tile_pool(name="ppq", bufs=6, space=bass.MemorySpace.PSUM) as ppq,
                tc.tile_pool(name="ptr", bufs=2, space=bass.MemorySpace.PSUM) as ptrp,
            ):
                # Startup DMA balanced across all three queues (~120 GB/s
                # each), graded need-order: small early pieces start matmuls
                # ~15us in, larger later pieces keep aggregate bandwidth up.
                xts0 = [pxt.tile([128, DC, 128], bf16, tag="xt", name="xt0")
                        for _ in range(3)]
                nc.scalar.dma_start(xts0[0][:, 0:14, :], xT_d[:, 0, 0:14, :])
                nc.sync.dma_start(xts0[0][:, 14:28, :], xT_d[:, 0, 14:28, :])
                nc.scalar.dma_start(wq_p[0][:, 0:2, :], wq_d[:, 0:2, :])
                nc.sync.dma_start(wq_p[0][:, 4:7, :], wq_d[:, 4:7, :])
                nc.scalar.dma_start(wq_p[0][:, 2:4, :], wq_d[:, 2:4, :])
                nc.gpsimd.dma_start(xts0[1], xT_d[:, 1, :, :])
                nc.sync.dma_start(wq_p[1][:, 4:7, :], wq_d[:, 11:14, :])
                nc.scalar.dma_start(wq_p[1][:, 0:4, :], wq_d[:, 7:11, :])
                nc.sync.dma_start(wq_p[2][:, 4:7, :], wq_d[:, 18:21, :])
                nc.scalar.dma_start(wq_p[2][:, 0:4, :], wq_d[:, 14:18, :])
                nc.gpsimd.dma_start(wq_p[3], wq_d[:, 21:28, :])
                nc.sync.dma_start(xts0[2], xT_d[:, 2, :, :])
                nc.gpsimd.dma_start(qsc, qsc_d[:])
                nc.gpsimd.dma_start(ksc, ksc_d[:])
                # masks/constants are needed late: keep them off the gpsimd
                # queue head so xt1's DMA issues first
                nc.gpsimd.dma_start(mdiag, mdiag_d[:])
                nc.gpsimd.dma_start(medge, medge_d[:])
                nc.gpsimd.memset(epsb, EPS)
                nc.gpsimd.memset(epsb2, EPS / (SCALE * SCALE))
                nc.gpsimd.memset(vS[:, :, :, 256:257], 1.0)

                def load_cs(tb):
                    cst = scr.tile([128, 128], bf16, tag="cs", bufs=3)
                    snt = scr.tile([128, 128], bf16, tag="sn", bufs=3)
                    nc.gpsimd.dma_start(cst, cos_d[:, tb, :])
                    nc.gpsimd.dma_start(snt, sin_d[:, tb, :])
                    return cst, snt

                def proj_epilogue(pq, j, cst, snt, scv, dstT, slot, tb, qmode):
                    """rmsnorm+rope head j of psum pq -> DMA-transpose into
                    dstT[:, hc, slot, tb]. qmode folds SCALE into the rstd so
                    qT carries SCALE/rms(q); k carries 1/rms(k)."""
                    sq = scr.tile([128, H], f32, tag="sq", bufs=1)
                    ssq = scr.tile([128, 1], f32, tag="ssq")
                    nc.scalar.activation(sq, pq[:, j, :], AF.Square, accum_out=ssq)
                    std = scr.tile([128, 1], f32, tag="std")
                    if qmode:
                        nc.scalar.activation(std, ssq, AF.Sqrt, bias=epsb2[:, 0:1],
                                             scale=1.0 / (H * SCALE * SCALE))
                    else:
                        nc.scalar.activation(std, ssq, AF.Sqrt, bias=epsb[:, 0:1],
                                             scale=1.0 / H)
                    rstd = scr.tile([128, 1], f32, tag="rstd")
                    nc.vector.reciprocal(rstd, std)
                    rb = rstd[:, 0:1].to_broadcast((128, 128))
                    x1 = pq[:, j, 0:128]
                    x2 = pq[:, j, 128:256]
                    t1 = scr.tile([128, 128], f32, tag="t1", bufs=2)
                    t2 = scr.tile([128, 128], f32, tag="t2", bufs=2)
                    qr = scr.tile([128, H], bf16, tag="qr", bufs=2)
                    nc.vector.tensor_mul(t1, x1, cst)
                    nc.vector.tensor_mul(t2, x2, snt)
                    nc.vector.tensor_sub(t1, t1, t2)
                    nc.vector.tensor_mul(qr[:, 0:128], t1, rb)
                    nc.vector.tensor_mul(t1, x2, cst)
                    nc.vector.tensor_mul(t2, x1, snt)
                    nc.vector.tensor_add(t1, t1, t2)
                    nc.vector.tensor_mul(qr[:, 128:256], t1, rb)
                    for hc in range(HC):
                        ptr = ptrp.tile([128, 1024], bf16, tag="ptr")
                        nc.tensor.transpose(ptr[:, 0:128],
                                            qr[:, hc * 128:(hc + 1) * 128], ident)
                        nc.vector.tensor_mul(
                            dstT[:, hc, slot, tb * 128:(tb + 1) * 128],
                            ptr[:, 0:128],
                            scv[:, hc:hc + 1].to_broadcast((128, 128)))

                def proj_block(tb, w_p, xt=None):
                    if xt is None:
                        xt = pxt.tile([128, DC, 128], bf16, tag="xt")
                        nc.gpsimd.dma_start(xt, xT_d[:, tb, :, :])
                    pqa = ppq.tile([128, 2, H], f32, tag="pq", name="pqa")
                    pqb = ppq.tile([128, 2, H], f32, tag="pq", name="pqb")
                    for dc in range(DC):
                        lhsT = xt[:, dc, :]
                        wt = w_p[dc // DCQ]
                        dcl = dc % DCQ
                        nc.tensor.matmul(pqa[:, :, :], lhsT, wt[:, dcl, 0:512],
                                         start=(dc == 0), stop=(dc == DC - 1))
                        nc.tensor.matmul(pqb[:, :, :], lhsT, wt[:, dcl, 512:1024],
                                         start=(dc == 0), stop=(dc == DC - 1))
                    return (pqa, pqb)

                # --- 1a: Q.  First two blocks run quarter-major so matmuls
                # start as soon as wq quarter 0 lands (instead of all of wq).
                cs01 = [load_cs(0), load_cs(1)]
                pqg = []
                for tbe in range(2):
                    pqa = ppq.tile([128, 2, H], f32, tag="pq", name="pqa")
                    pqb = ppq.tile([128, 2, H], f32, tag="pq", name="pqb")
                    pqg.append((pqa, pqb))
                # Dummy matmuls keep the PE's HAM activity monitor warm while
                # the startup DMAs land, so the first-group matmuls run at
                # 2.4 GHz instead of the cold 1.2 GHz default.
                warm = ptrp.tile([128, 512], f32, tag="ptr", name="warm")

                def emit_warm(n):
                    for _ in range(n):
                        nc.tensor.matmul(warm[:, 0:128], ident, ident,
                                         start=True, stop=True)

                emit_warm(32)
                for qi in range(4):
                    for tbe in range(2):
                        for dcl in range(DCQ):
                            dc = qi * DCQ + dcl
                            lhsT = xts0[tbe][:, dc, :]
                            nc.tensor.matmul(pqg[tbe][0][:, :, :], lhsT,
                                             wq_p[qi][:, dcl, 0:512],
                                             start=(dc == 0), stop=(dc == DC - 1))
                            nc.tensor.matmul(pqg[tbe][1][:, :, :], lhsT,
                                             wq_p[qi][:, dcl, 512:1024],
                                             start=(dc == 0), stop=(dc == DC - 1))
                            if qi < 2:
                                emit_warm(2)
                wkv_p = [wbp.tile([128, DCQ, 2 * KVH * H], bf16, tag="wb",
                                  name=f"wkv_{qi}") for qi in range(4)]
                for tbe in range(2):
                    cst, snt = cs01[tbe]
                    for jh in range(QH):
                        proj_epilogue(pqg[tbe][jh // 2], jh % 2, cst, snt, qsc,
                                      qT, jh, tbe, qmode=True)
                for tb in range(2, TBN):
                    cst, snt = load_cs(tb)
                    pq2 = proj_block(tb, wq_p, xt=(xts0[2] if tb == 2 else None))
                    if tb == 7:
                        # paced wkv prefetch behind the xt stream
                        for qi in range(4):
                            nc.gpsimd.dma_start(wkv_p[qi],
                                                wkv_d[:, qi * 7:(qi + 1) * 7, :])
                    for jh in range(QH):
                        proj_epilogue(pq2[jh // 2], jh % 2, cst, snt, qsc,
                                      qT, jh, tb, qmode=True)

                # --- 1b: K and V.  wo recycles the wq slots (WAR on wq's
                # last read at the end of 1a; needed only at phase 2).
                wo_p = [wap.tile([128, OC // 4, D], bf16, tag="wa", name=f"wo_{qi}")
                        for qi in range(4)]
                for qi in range(4):
                    nc.scalar.dma_start(
                        wo_p[qi], wo_d[:, qi * (OC // 4):(qi + 1) * (OC // 4), :])
                for tb in range(TBN):
                    cst, snt = load_cs(tb)
                    pq2 = proj_block(tb, wkv_p)
                    if tb >= TBN - 2:
                        # park the K psum in SBUF with two quick copies so the
                        # serial epilogue chain doesn't block the PE FIFO or
                        # the phase-2 PSUM pools; the epilogue itself is
                        # deferred into phase 2.
                        for kv in range(KVH):
                            nc.scalar.activation(kqP[:, tb - (TBN - 2), kv, :],
                                                 pq2[0][:, kv, :], AF.Copy)
                    else:
                        for kv in range(KVH):
                            proj_epilogue(pq2[0], kv, cst, snt, ksc, kT, kv, tb,
                                          qmode=False)
                    for kv in range(KVH):
                        nc.scalar.activation(vS[:, tb, kv, 0:256],
                                             pq2[1][:, kv, :], AF.Copy)


            # ---------------- phase 2: attention + o-proj ----------------
            with (
                tc.tile_pool(name="att", bufs=2) as att,
                tc.tile_pool(name="psA", bufs=3, space=bass.MemorySpace.PSUM) as psA,
                tc.tile_pool(name="po", bufs=3, space=bass.MemorySpace.PSUM) as pop,
                tc.tile_pool(name="py", bufs=2, space=bass.MemorySpace.PSUM) as pyp,
            ):
                # keep the PE warm across the phase boundary (the 1b tail
                # epilogue chain leaves the PE idle long enough to re-throttle)
                warm2 = psA.tile([128, 512], f32, tag="pb", name="warm2")
                for _ in range(16):
                    nc.tensor.matmul(warm2[:, 0:128], ident, ident,
                                     start=True, stop=True)

                for tb in range(TBN):
                    sb0 = max(0, tb - WB)
                    ns = tb - sb0 + 1
                    outsb = att.tile([128, QH, H], bf16, tag="outsb")
                    outT = att.tile([128, OC, 128], bf16, tag="outT")

                    def emit_norm(kvi, po0, po1, use_act=False):
                        for jj, po in ((0, po0), (1, po1)):
                            recip = att.tile([128, 1], f32, tag="recip", bufs=4)
                            nc.vector.reciprocal(recip, po[:, 256:257])
                            if use_act:
                                nc.scalar.activation(
                                    outsb[:, 2 * kvi + jj, :], po[:, 0:256],
                                    AF.Copy, scale=recip[:, 0:1])
                            else:
                                nc.vector.tensor_mul(
                                    outsb[:, 2 * kvi + jj, :], po[:, 0:256],
                                    recip[:, 0:1].to_broadcast((128, 256)))

                    def emit_tp(c):
                        ptr = pyp.tile([128, 1024], bf16, tag="py", name="ptr")
                        nc.tensor.transpose(
                            ptr[:, 0:128],
                            outsb[:, c // 2, (c % 2) * 128:(c % 2 + 1) * 128], ident)
                        if c % 2 == 0:
                            nc.vector.tensor_copy(outT[:, c, :], ptr[:, 0:128])
                        else:
                            nc.scalar.activation(outT[:, c, :], ptr[:, 0:128],
                                                 AF.Copy)

                    po_all = []
                    tpc = 0
                    npairs0 = (ns + 1) // 2
                    for kvi in range(KVH):
                        po0 = pop.tile([128, 512], f32, tag="po", name="po0")
                        po1 = pop.tile([128, 512], f32, tag="po", name="po1")
                        eTp = [None] * npairs0

                        def emit_pv(s, po0=po0, po1=po1, kvi=kvi, eTp=eTp):
                            st = (s == 0)
                            sp = (s == ns - 1)
                            e2 = eTp[s // 2]
                            off = (s % 2) * 256
                            nc.tensor.matmul(po0[:, 0:257], e2[:, off:off + 128],
                                             vS[:, sb0 + s, kvi, 0:257],
                                             start=st, stop=sp)
                            nc.tensor.matmul(po1[:, 0:257],
                                             e2[:, off + 128:off + 256],
                                             vS[:, sb0 + s, kvi, 0:257],
                                             start=st, stop=sp)

                        def emit_qk_pair(p, kvi=kvi, eTp=eTp):
                            # two s-blocks' logitsT side by side in one PSUM
                            # bank; one Exp covers both.
                            s_lo = 2 * p
                            blocks = [s_lo] + ([s_lo + 1] if s_lo + 1 < ns else [])
                            pl = psA.tile([128, 512], f32, tag="pb", name="pl")
                            for si, s in enumerate(blocks):
                                for hc in range(HC):
                                    nc.tensor.matmul(
                                        pl[:, si * 256:si * 256 + 256],
                                        kT[:, hc, kvi,
                                           (sb0 + s) * 128:(sb0 + s + 1) * 128],
                                        qT[:, hc, 2 * kvi:2 * kvi + 2,
                                           tb * 128:(tb + 1) * 128],
                                        start=(hc == 0), stop=(hc == HC - 1))
                                if s == ns - 1:  # causal diagonal block
                                    nc.vector.tensor_add(
                                        pl[:, si * 256:si * 256 + 128],
                                        pl[:, si * 256:si * 256 + 128], mdiag)
                                    nc.vector.tensor_add(
                                        pl[:, si * 256 + 128:si * 256 + 256],
                                        pl[:, si * 256 + 128:si * 256 + 256], mdiag)
                                if tb >= WB and s == 0:  # window edge block
                                    nc.vector.tensor_add(
                                        pl[:, si * 256:si * 256 + 128],
                                        pl[:, si * 256:si * 256 + 128], medge)
                                    nc.vector.tensor_add(
                                        pl[:, si * 256 + 128:si * 256 + 256],
                                        pl[:, si * 256 + 128:si * 256 + 256], medge)
                            w = 256 * len(blocks)
                            e2 = att.tile([128, 512], bf16, tag="eT", bufs=4)
                            nc.scalar.activation(e2[:, 0:w], pl[:, 0:w], AF.Exp)
                            eTp[p] = e2

                        npairs = (ns + 1) // 2
                        for p in range(min(2, npairs)):
                            emit_qk_pair(p)
                        for p in range(2, npairs):
                            emit_qk_pair(p)
                            for s in (2 * p - 4, 2 * p - 3):
                                emit_pv(s)
                        for s in range(max(0, 2 * npairs - 4), ns):
                            emit_pv(s)
                        po_all.append((po0, po1))
                        if kvi == 0:
                            emit_norm(0, po0, po1)
                    while tpc < 4:
                        emit_tp(tpc)
                        tpc += 1
                    emit_norm(1, po_all[1][0], po_all[1][1], use_act=True)
                    for dx in range(NDOUT):
                        py = pyp.tile([128, 512], f32, tag="py")
                        for c in range(OC):
                            if dx == 0 and c == 4:
                                # kv1 head transposes land while c0..c3 stream
                                for cc in range(4, OC):
                                    emit_tp(cc)
                            nc.tensor.matmul(py, outT[:, c, :],
                                             wo_p[c // 2][:, c % 2,
                                                          dx * 512:(dx + 1) * 512],
                                             start=(c == 0), stop=(c == OC - 1))
                        ysb = att.tile([128, 512], bf16, tag="ysb", bufs=3)
                        if dx % 2 == 0:
                            nc.vector.tensor_copy(ysb, py)
                        else:
                            nc.scalar.activation(ysb, py, AF.Copy)
                        oeng = nc.sync if dx % 2 == 0 else nc.scalar
                        oeng.dma_start(
                            out_d[tb * 128:(tb + 1) * 128, dx * 512:(dx + 1) * 512],
                            ysb)
                    if tb <= 1:
                        # deferred kT(14/15) epilogue from the parked SBUF
                        # copy; its latency hides under tb0/tb1's o-proj.
                        pidx = tb
                        tbv = TBN - 2 + pidx
                        cst15 = att.tile([128, 128], bf16, tag="cs2")
                        snt15 = att.tile([128, 128], bf16, tag="cs2")
                        nc.gpsimd.dma_start(cst15, cos_d[:, tbv, :])
                        nc.gpsimd.dma_start(snt15, sin_d[:, tbv, :])
                        for kv in range(KVH):
                            sq = att.tile([128, H], f32, tag="sq2", bufs=1)
                            ssq = att.tile([128, 1], f32, tag="ss2")
                            nc.scalar.activation(sq, kqP[:, pidx, kv, :],
                                                 AF.Square, accum_out=ssq)
                            std = att.tile([128, 1], f32, tag="st2")
                            nc.scalar.activation(std, ssq, AF.Sqrt,
                                                 bias=epsb[:, 0:1], scale=1.0 / H)
                            rstd = att.tile([128, 1], f32, tag="rs2")
                            nc.vector.reciprocal(rstd, std)
                            rb = rstd[:, 0:1].to_broadcast((128, 128))
                            x1 = kqP[:, pidx, kv, 0:128]
                            x2 = kqP[:, pidx, kv, 128:256]
                            t1 = att.tile([128, 128], f32, tag="t1b")
                            t2 = att.tile([128, 128], f32, tag="t2b")
                            qr = att.tile([128, H], bf16, tag="qr2")
                            nc.vector.tensor_mul(t1, x1, cst15)
                            nc.vector.tensor_mul(t2, x2, snt15)
                            nc.vector.tensor_sub(t1, t1, t2)
                            nc.vector.tensor_mul(qr[:, 0:128], t1, rb)
                            nc.vector.tensor_mul(t1, x2, cst15)
                            nc.vector.tensor_mul(t2, x1, snt15)
                            nc.vector.tensor_add(t1, t1, t2)
                            nc.vector.tensor_mul(qr[:, 128:256], t1, rb)
                            for hc in range(HC):
                                ptr = pyp.tile([128, 1024], bf16, tag="py",
                                               name="ptr15")
                                nc.tensor.transpose(
                                    ptr[:, 0:128],
                                    qr[:, hc * 128:(hc + 1) * 128], ident)
                                nc.vector.tensor_mul(
                                    kT[:, hc, kv, tbv * 128:(tbv + 1) * 128],
                                    ptr[:, 0:128],
                                    ksc[:, hc:hc + 1].to_broadcast((128, 128)))

    nc.compile()
    return nc


def _tile128(a):
    """[128*n, m] -> [128, n, m] with row index = chunk*128 + partition."""
    n = a.shape[0] // 128
    return np.ascontiguousarray(
        a.reshape(n, 128, *a.shape[1:]).transpose(1, 0, *range(2, a.ndim + 1)))


def _rope_tabs():
    j = np.arange(128, dtype=np.float64)
    ts = ROPE_BASE ** (2.0 * j / H)
    ang = np.arange(T, dtype=np.float64)[:, None] / ts[None, :]
    return (_tile128(np.cos(ang).astype(np.float32).astype(BF16)),
            _tile128(np.sin(ang).astype(np.float32).astype(BF16)))


def kernel(x, w_q, w_kv, w_o, q_norm_scale, k_norm_scale):
    from concourse.bass_utils import run_bass_kernel_spmd

    if "nc" not in _cached:
        _cached["nc"] = _build()
    nc = _cached["nc"]

    x = np.asarray(x, np.float32)
    w_q = np.asarray(w_q, np.float32)
    w_kv = np.asarray(w_kv, np.float32)
    w_o = np.asarray(w_o, np.float32)
    cos_t, sin_t = _rope_tabs()
    qsc = np.ascontiguousarray(
        np.asarray(q_norm_scale, np.float32).reshape(HC, 128).T)
    ksc = np.ascontiguousarray(
        np.asarray(k_norm_scale, np.float32).reshape(HC, 128).T)

    p = np.arange(128)[:, None]   # s offset (partitions)
    f = np.arange(128)[None, :]   # t offset (free)
    mdiag = np.where(f >= p, 0.0, NEG).astype(np.float32)      # causal: t >= s
    medge = np.where(p >= f + 1, 0.0, NEG).astype(np.float32)  # window: s >= t+1

    xT_b = []
    for b in range(B):
        xt = _tile128(np.ascontiguousarray(x[b].T).astype(BF16))  # [128, DC, T]
        xT_b.append(np.ascontiguousarray(
            xt.reshape(128, DC, TBN, 128).transpose(0, 2, 1, 3)))

    in_maps = []
    for c in range(8):
        b, kp = c // 4, c % 4
        n0, k0 = 4 * kp, 2 * kp
        wq = _tile128(w_q[n0:n0 + 4].transpose(1, 0, 2).reshape(D, QH * H).astype(BF16))
        wk = w_kv[0, k0:k0 + 2].transpose(1, 0, 2).reshape(D, KVH * H)
        wv = w_kv[1, k0:k0 + 2].transpose(1, 0, 2).reshape(D, KVH * H)
        wkv = _tile128(np.concatenate([wk, wv], axis=1).astype(BF16))
        wo = _tile128(w_o[n0:n0 + 4].reshape(QH * H, D).astype(BF16))
        m = {"xT": xT_b[b], "wq": wq, "wkv": wkv, "wo": wo,
             "mdiag": mdiag, "medge": medge,
             "cos": cos_t, "sin": sin_t, "qsc": qsc, "ksc": ksc}
        in_maps.append(m)

    res = run_bass_kernel_spmd(nc, in_maps, core_ids=list(range(8)))
    _cached["last_result"] = res
    y = np.zeros((B, T, D), np.float32)
    for c in range(8):
        y[c // 4] += np.asarray(res.results[c]["out"], np.float32)
    return y
